# revision 17
# baseline (speedup 1.0000x reference)
"""AttentionBlock kernel for 8 Trainium2 NeuronCores.

Problem: x[4,128,64,64] -> GroupNorm(8) -> 1x1 conv QKV -> full self-attention
over 4096 tokens per batch -> output proj -> residual.

Sharding: 8 cores = 4 batches x 2 row-halves of the attention matrix.
Each core gets its batch's full x (token-rolled so that the SPMD program
always computes attention rows 0..2047 of its input; softmax over keys is
permutation-invariant), redundantly computes groupnorm+QKV (cheap), and
computes its 2048-row slice of attention against full K/V. No collectives.

Layout: feature-major [C=128 partitions, tokens free] for h/q/k.
  S^T[key,row] = matmul(lhsT=k[:,kc], rhs=q[:,win])     (keys on psum partitions)
  exp fused with psum eviction on ScalarE (no max subtraction needed:
  scores are ~N(0,1), exp is safe in fp32)
  V is computed directly token-major: v_tm[tok,Cout] = matmul(lhsT=h[:,tok128],
  rhs=wvT), with an appended ones column so the softmax denominator Z
  falls out of the A@V matmul:
  A@V: out[row, C+1] += matmul(lhsT=expS^T[kc,rc], rhs=v_aug[kc])
Window epilogues (normalize/transpose/proj/residual) are software-pipelined
into the next window's key-chunk groups to keep PE and ACT streaming.
"""

import numpy as np
import ml_dtypes

import concourse.bass as bass
import concourse.mybir as mybir
import concourse.tile as tile
from concourse import bacc
from concourse.bass_utils import run_bass_kernel_spmd

F32 = mybir.dt.float32
BF16 = mybir.dt.bfloat16
AF = mybir.ActivationFunctionType
OP = mybir.AluOpType

B = 4
C = 128
HW = 4096
ROWS = 2048          # attention rows computed per core
WIN = 512            # row window
NWIN = ROWS // WIN
KC = HW // 128       # 32 key chunks
G = 2                # key chunks per S^T psum tile / exp instruction
NG = KC // G
EPS = 1e-5
SCALE = float(1.0 / np.sqrt(C))


def _flat(ap):
    return ap.rearrange("p a b -> p (a b)")


def _body(tc):
    nc = tc.nc
    xin = nc.dram_tensor("xin", [C, HW], F32, kind="ExternalInput").ap()
    qkvw = nc.dram_tensor("qkvw", [C, 3 * C], F32, kind="ExternalInput").ap()
    projw = nc.dram_tensor("projw", [C, C], F32, kind="ExternalInput").ap()
    # packed per-channel vectors: cols 0-2 qkv bias (q,k,v), 3 proj_b, 4 norm_w, 5 norm_b
    vecs = nc.dram_tensor("vecs", [C, 6], F32, kind="ExternalInput").ap()
    # host-precomputed block-diag ones(16)/16 for group stat aggregation
    gmat = nc.dram_tensor("gmat", [C, C], F32, kind="ExternalInput").ap()
    ident = nc.dram_tensor("ident", [C, C], BF16, kind="ExternalInput").ap()
    # v bias broadcast across rows: vbb[r, c] = qkv_b[2C + c]
    vbb = nc.dram_tensor("vbb", [C, C], F32, kind="ExternalInput").ap()
    # v bias broadcast across rows: vbb[r, c] = qkv_b[2C + c]
    out = nc.dram_tensor("out", [C, ROWS], F32, kind="ExternalOutput").ap()
    warm = nc.dram_tensor("warm", [C, 4], F32, kind="ExternalOutput").ap()

    with (
        tc.tile_pool(name="const", bufs=1) as const,
        tc.tile_pool(name="big", bufs=1) as big,
        tc.tile_pool(name="gn", bufs=1) as gn,
        tc.tile_pool(name="work", bufs=3) as work,
        tc.tile_pool(name="ep", bufs=6) as ep,
        tc.tile_pool(name="outp", bufs=3) as outp,
        tc.tile_pool(name="psum_st", bufs=2, space="PSUM") as psum_st,
        tc.tile_pool(name="psum_av", bufs=2, space="PSUM") as psum_av,
        tc.tile_pool(name="psum_misc", bufs=2, space="PSUM") as psum_misc,
    ):
        # ---- small consts first (cheap, unblock downstream chains), then x ----
        vecs_sb = const.tile([C, 6], F32)
        nc.sync.dma_start(vecs_sb[:], vecs)
        gmat_sb = const.tile([C, C], F32)
        nc.gpsimd.dma_start(gmat_sb[:], gmat)
        ident_sb = const.tile([C, C], BF16)
        nc.gpsimd.dma_start(ident_sb[:], ident)
        qkvw_f = const.tile([C, 3 * C], F32)
        nc.gpsimd.dma_start(qkvw_f[:], qkvw)
        projw_f = const.tile([C, C], F32)
        nc.gpsimd.dma_start(projw_f[:], projw)
        vbb_sb = const.tile([C, C], F32)
        nc.gpsimd.dma_start(vbb_sb[:], vbb)
        x_sb = big.tile([C, HW], F32)
        for c in range(4):
            nc.sync.dma_start(x_sb[:, c * 1024:(c + 1) * 1024],
                              xin[:, c * 1024:(c + 1) * 1024])
        qkvw_bf = const.tile([C, 3 * C], BF16)
        nc.vector.tensor_copy(qkvw_bf[:], qkvw_f[:])
        projw_bf = const.tile([C, C], BF16)
        nc.vector.tensor_copy(projw_bf[:], projw_f[:])
        zeros_sb = const.tile([C, 512], BF16)
        nc.vector.memset(zeros_sb[:], 0.0)
        # warm the PE clock (HAM) during the x-DMA wait with dummy matmuls;
        # anchored by a tiny DRAM store so DCE keeps them
        wp = psum_misc.tile([C, 512], F32, tag="misc")
        for i in range(36):
            nc.tensor.matmul(wp[:], lhsT=zeros_sb[:, :128], rhs=zeros_sb[:],
                             start=True, stop=True)
        warm_sb = const.tile([C, 4], F32)
        nc.vector.tensor_copy(warm_sb[:], wp[:, 0:4])
        nc.sync.dma_start(warm, warm_sb[:])
        eps_sb = gn.tile([C, 1], F32)
        nc.vector.memset(eps_sb[:], EPS)
        # prefetch the sqrt ACT table set while DMAs run
        scr0 = gn.tile([C, 1], F32)
        nc.scalar.activation(scr0[:], eps_sb[:], AF.Sqrt)

        stats = gn.tile([C, 8, 6], F32)
        for c in range(8):
            nc.vector.bn_stats(stats[:, c, :], x_sb[:, c * 512:(c + 1) * 512])
        # keep the PE HAM-warm across the stats chain (reads x, so these run
        # after the DMA lands); fp32 matmuls ~850ns each
        wp2 = psum_misc.tile([C, 512], F32, tag="misc")
        for i in range(4):
            nc.tensor.matmul(wp2[:], lhsT=gmat_sb[:], rhs=x_sb[:, i * 512:(i + 1) * 512],
                             start=True, stop=True)
        warm2_sb = gn.tile([C, 4], F32)
        nc.vector.tensor_copy(warm2_sb[:], wp2[:, 0:4])
        nc.sync.dma_start(warm[:, 0:4].rearrange("a b -> a b"), warm2_sb[:])
        mv = gn.tile([C, 2], F32)
        nc.vector.bn_aggr(mv[:], stats[:])
        # e2: col0 = mean_c, col1 = var_c + mean_c^2
        e2 = gn.tile([C, 2], F32)
        nc.vector.tensor_copy(e2[:, 0:1], mv[:, 0:1])
        nc.vector.tensor_tensor(e2[:, 1:2], mv[:, 0:1], mv[:, 0:1], OP.mult)
        nc.vector.tensor_tensor(e2[:, 1:2], e2[:, 1:2], mv[:, 1:2], OP.add)
        # per-channel group stats via block-diag matmul (gmat includes /16)
        gs = psum_misc.tile([C, 2], F32, tag="misc")
        nc.tensor.matmul(gs[:], lhsT=gmat_sb[:], rhs=e2[:], start=True, stop=True)
        gsb = gn.tile([C, 2], F32)
        nc.vector.tensor_copy(gsb[:], gs[:])
        # var_g = E2_g - mean_g^2 ; rstd = 1/sqrt(var+eps)
        msq = gn.tile([C, 1], F32)
        nc.vector.tensor_tensor(msq[:], gsb[:, 0:1], gsb[:, 0:1], OP.mult)
        var = gn.tile([C, 1], F32)
        nc.vector.tensor_tensor(var[:], gsb[:, 1:2], msq[:], OP.subtract)
        std = gn.tile([C, 1], F32)
        nc.scalar.activation(std[:], var[:], AF.Sqrt, bias=eps_sb[:])
        # prefetch the exp ACT table set now (dep on std orders it after Sqrt)
        scr1 = gn.tile([C, 1], F32)
        nc.scalar.activation(scr1[:], std[:], AF.Exp)
        rstd = gn.tile([C, 1], F32)
        nc.vector.reciprocal(rstd[:], std[:])
        scl = gn.tile([C, 1], F32)
        nc.vector.tensor_tensor(scl[:], vecs_sb[:, 4:5], rstd[:], OP.mult)
        tmp = gn.tile([C, 1], F32)
        nc.vector.tensor_tensor(tmp[:], gsb[:, 0:1], scl[:], OP.mult)
        shf = gn.tile([C, 1], F32)
        nc.vector.tensor_tensor(shf[:], vecs_sb[:, 5:6], tmp[:], OP.subtract)
        # h = x*scale + shift  (bf16 for the matmuls)
        h_sb = big.tile([C, HW], BF16)
        for c in range(4):
            nc.vector.tensor_scalar(
                out=h_sb[:, c * 1024:(c + 1) * 1024],
                in0=x_sb[:, c * 1024:(c + 1) * 1024],
                scalar1=scl[:], scalar2=shf[:], op0=OP.mult, op1=OP.add)

        # ---- qkv projections (feature-major), bias fused into eviction.
        # Only the tiles needed by window 0's first groups are emitted up
        # front; the rest are woven into window 0's group loop to keep the
        # scalar engine (softmax exp) streaming as early as possible.
        q_sb = big.tile([C, ROWS], BF16)
        k_sb = big.tile([C, HW], BF16)
        v_aug = big.tile([C, KC, 129], BF16)
        nc.vector.memset(v_aug[:, :, 128:129], 1.0)

        def emit_qkv_tile(t, dst, c2):
            ps = psum_st.tile([C, 2, 512], F32, tag="st")
            for j in range(2):
                nc.tensor.matmul(
                    ps[:, j, :],
                    lhsT=qkvw_bf[:, t * C:(t + 1) * C],
                    rhs=h_sb[:, (c2 * 2 + j) * 512:(c2 * 2 + j + 1) * 512],
                    start=True, stop=True)
            # evict halves on separate engines (ACT idle during setup)
            nc.vector.tensor_scalar(
                out=dst[:, c2 * 1024:c2 * 1024 + 512],
                in0=ps[:, 0, :], scalar1=vecs_sb[:, t:t + 1], scalar2=None,
                op0=OP.add)
            nc.scalar.activation(
                dst[:, c2 * 1024 + 512:(c2 + 1) * 1024], ps[:, 1, :],
                AF.Identity, bias=vecs_sb[:, t:t + 1])

        def emit_v_tm(kc):
            # token-major v chunk directly from PE: v_tm[tok, cout]
            vp = psum_misc.tile([C, C], F32, tag="misc")
            nc.tensor.matmul(vp[:], lhsT=h_sb[:, kc * 128:(kc + 1) * 128],
                             rhs=qkvw_bf[:, 2 * C:3 * C], start=True, stop=True)
            nc.vector.tensor_tensor(v_aug[:, kc, 0:128], vp[:], vbb_sb[:], OP.add)

        for t, dst, c2 in [(1, k_sb, 0), (0, q_sb, 0), (1, k_sb, 1),
                           (0, q_sb, 1), (1, k_sb, 2), (1, k_sb, 3)]:
            emit_qkv_tile(t, dst, c2)
        for kc in range(KC):
            emit_v_tm(kc)

        # ---- attention with software-pipelined window epilogues ----
        def ep_normalize(avs):
            aos = []
            for rc in range(4):
                sl = avs[rc // 2][:, rc % 2, :]
                rz = ep.tile([C, 1], F32, tag="rz")
                nc.vector.reciprocal(rz[:], sl[:, 128:129])
                ao = ep.tile([C, C], BF16, tag="ao")
                nc.vector.tensor_scalar_mul(ao[:], sl[:, 0:128], rz[:])
                aos.append(ao)
            return aos

        def ep_step(state, step):
            w, aos, attn_fm = state
            if step < 4:
                rc = step
                tp = psum_misc.tile([C, C], BF16, tag="misc")
                nc.tensor.transpose(tp[:], aos[rc][:], ident_sb[:])
                nc.vector.tensor_copy(attn_fm[:, rc * 128:(rc + 1) * 128], tp[:])
            else:
                pj = psum_misc.tile([C, 512], F32, tag="misc")
                nc.tensor.matmul(pj[:], lhsT=projw_bf[:], rhs=attn_fm[:],
                                 start=True, stop=True)
                o = outp.tile([C, WIN], F32, tag="o")
                nc.vector.tensor_tensor(o[:], pj[:],
                                        x_sb[:, w * WIN:(w + 1) * WIN], OP.add)
                nc.vector.tensor_scalar(out=o[:], in0=o[:],
                                        scalar1=vecs_sb[:, 3:4],
                                        scalar2=None, op0=OP.add)
                nc.sync.dma_start(out[:, w * WIN:(w + 1) * WIN], o[:])

        pend = None
        for w in range(NWIN):
            avs = []
            for _ in range(2):
                av = psum_av.tile([C, 2, 129], F32, tag="av")
                # zero-fill whole region + set has_written via one matmul
                nc.tensor.matmul(av[:], lhsT=zeros_sb[:, :128],
                                 rhs=zeros_sb[:, :258],
                                 start=True, stop=False, skip_group_check=True)
                avs.append(av)

            # S^T is emitted one group ahead of exp/A@V so the PE never
            # head-of-line blocks on the current group's exp
            sts = {}

            def emit_st(w, g):
                st = psum_st.tile([C, G, 512], F32, tag="st")
                mm = None
                for j in range(G):
                    kc = g * G + j
                    mm = nc.tensor.matmul(
                        st[:, j, :],
                        lhsT=k_sb[:, kc * 128:(kc + 1) * 128],
                        rhs=q_sb[:, w * WIN:(w + 1) * WIN],
                        start=True, stop=True)
                sts[g] = (st, mm)

            emit_st(w, 0)
            for g in range(NG):
                if g + 1 < NG:
                    emit_st(w, g + 1)
                st, _ = sts.pop(g)
                ex = work.tile([C, G, 512], BF16, tag="ex")
                act = nc.scalar.activation(_flat(ex[:]), _flat(st[:]), AF.Exp,
                                           scale=SCALE)
                for j in range(G):
                    kc = g * G + j
                    for rc in range(4):
                        nc.tensor.matmul(
                            avs[rc // 2][:, rc % 2, :],
                            lhsT=ex[:, j, rc * 128:(rc + 1) * 128],
                            rhs=v_aug[:, kc, 0:129],
                            start=False, stop=(kc == KC - 1),
                            skip_group_check=True)
                if pend is not None and 1 <= g <= 5:
                    ep_step(pend, g - 1)
            aos = ep_normalize(avs)
            attn_fm = outp.tile([C, WIN], BF16, tag="attn_fm")
            pend = (w, aos, attn_fm)
        for step in range(5):
            ep_step(pend, step)


_NC_CACHE = None


def _get_nc():
    global _NC_CACHE
    if _NC_CACHE is None:
        nc = bacc.Bacc("TRN2", target_bir_lowering=False, debug=False,
                       num_devices=8)
        with tile.TileContext(nc) as tc:
            _body(tc)
        nc.compile()
        _NC_CACHE = nc
    return _NC_CACHE


def _make_in_maps(x, norm_w, norm_b, qkv_w, qkv_b, proj_w, proj_b):
    x = np.ascontiguousarray(np.asarray(x, np.float32)).reshape(B, C, HW)
    qkvw = np.ascontiguousarray(np.asarray(qkv_w, np.float32).T)      # [C, 3C]
    projw = np.ascontiguousarray(np.asarray(proj_w, np.float32).T)    # [C, C]
    qkv_b = np.asarray(qkv_b, np.float32)
    vecs = np.empty((C, 6), np.float32)
    vecs[:, 0:3] = qkv_b.reshape(3, C).T
    vecs[:, 3] = np.asarray(proj_b, np.float32)
    vecs[:, 4] = np.asarray(norm_w, np.float32)
    vecs[:, 5] = np.asarray(norm_b, np.float32)
    vbb = np.ascontiguousarray(
        np.broadcast_to(qkv_b[2 * C:3 * C][None, :], (C, C)).astype(np.float32))
    gmat = np.zeros((C, C), np.float32)
    for g in range(8):
        gmat[g * 16:(g + 1) * 16, g * 16:(g + 1) * 16] = 1.0 / 16.0
    ident = np.eye(C, dtype=ml_dtypes.bfloat16)
    shared = {"qkvw": qkvw, "projw": projw, "vecs": vecs, "gmat": gmat,
              "ident": ident, "vbb": vbb}
    in_maps = []
    for core in range(8):
        b, half = core // 2, core % 2
        xb = x[b]
        if half:
            xb = np.concatenate([xb[:, ROWS:], xb[:, :ROWS]], axis=1)
        in_maps.append({"xin": np.ascontiguousarray(xb), **shared})
    return in_maps


def _assemble(results):
    out = np.empty((B, C, HW), np.float32)
    for core in range(8):
        b, half = core // 2, core % 2
        out[b, :, half * ROWS:(half + 1) * ROWS] = results[core]["out"]
    return out.reshape(B, C, 64, 64)


def kernel(x, norm_w, norm_b, qkv_w, qkv_b, proj_w, proj_b):
    nc = _get_nc()
    in_maps = _make_in_maps(x, norm_w, norm_b, qkv_w, qkv_b, proj_w, proj_b)
    res = run_bass_kernel_spmd(nc, in_maps, core_ids=list(range(8)))
    return _assemble(res.results)
